# revision 17
# baseline (speedup 1.0000x reference)
"""Trainium2 Bass kernel for nn_AdaptativeGCN (gnn_message_passing).

Computation (reference):
    sec   = relu(A @ (X Ws1) + X Ws2 + bs)                 [N, 32]
    S     = [sec | P]                                      [N, 96]
    msec  = A @ (S Wm2a) + S Wm2b + bm2                    [N, 7]
    M     = [X | P]                                        [N, 192]
    main  = A @ (M Wma) + M Wmb + bm                       [N, 7]
    out   = softmax(0.5*(v2*main + v1*msec), axis=-1)      [N, 7]

Algebraic restructuring:
    0.5*(v2*main + v1*msec) = A @ Gc + L + bl  where
      Gc = 0.5*(v1*(S Wm2a) + v2*(M Wma))     [N, 7]   (one 7-col pass-2 product)
      L  = 0.5*(v1*(S Wm2b) + v2*(M Wmb))     [N, 7]   (local, no A)
      bl = 0.5*(v1*bm2 + v2*bm)               [7]

Distribution: row-shard A over 8 cores (1250 rows each). Host uploads
A[rows_c,:].T as fp8-e4m3 [10000, 1250] per core so the contraction dim lands
on SBUF partitions. The fp8 shard (11.9 MiB) is streamed from HBM once and
kept entirely resident in SBUF; pass 2 reuses it with zero HBM traffic. A@*
matmuls run fp8 DoubleRow (two k-planes per pass through the PE array). The
only cross-core dependency is Gc: one ~9 KB-per-core AllGather between the
passes. fp8 quantization error on the 10000-term contractions averages out:
host-simulated end-to-end rel err ~6e-4 vs the f32 reference.
"""

import sys
import types

import numpy as np


def _install_ntff_hook():
    """run_bass_kernel_spmd(trace=True) under axon needs antenv.axon_hooks,
    which the agent image lacks; register the ctypes-based hook ourselves."""
    try:
        from antenv.axon_hooks import get_axon_ntff_profile_hook  # noqa: F401
        return
    except ImportError:
        pass
    try:
        from trn_agent_boot.trn_boot import _ntff_profile_via_ctypes
        hook = _ntff_profile_via_ctypes('/opt/axon/libaxon_pjrt.so')
    except Exception:
        hook = None
    mod = types.ModuleType('antenv.axon_hooks')
    mod.get_axon_ntff_profile_hook = lambda: hook
    mod.set_axon_ntff_profile_hook = lambda h: None
    sys.modules['antenv.axon_hooks'] = mod


_install_ntff_hook()
if '/opt/trn_rl_repo' not in sys.path:
    sys.path.insert(0, '/opt/trn_rl_repo')

import ml_dtypes  # noqa: E402
import concourse.bacc as bacc  # noqa: E402
import concourse.mybir as mybir  # noqa: E402
from concourse import masks, tile  # noqa: E402
from concourse import bass_utils as _bu  # noqa: E402
from concourse.bass_utils import run_bass_kernel_spmd  # noqa: E402

import os  # noqa: E402

if os.environ.get("LDWOPT") == "1" and not getattr(_bu, "_ldw_patched", False):
    _orig_run_command = _bu.run_command

    def _patched_run_command(argv, **kwargs):
        argv = [a.replace("--enable-ldw-opt=false", "--enable-ldw-opt=true")
                if isinstance(a, str) else a for a in argv]
        return _orig_run_command(argv, **kwargs)

    _bu.run_command = _patched_run_command
    _bu._ldw_patched = True

BF16 = ml_dtypes.bfloat16
FP8 = ml_dtypes.float8_e4m3
NCORES = 8
N = 10000
F_T, F_P = 128, 64
SEC, MC = 32, 7
RL = N // NCORES            # local rows per core = 1250
RLP = 1280                  # padded SBUF pitch per AT k-tile (16B-aligned)
NT = (N + 127) // 128       # k-tiles over the contraction dim = 79
KW = [128] * (NT - 1) + [N - 128 * (NT - 1)]          # last = 16
CHUNKS = [(0, 512), (512, 512), (1024, RL - 1024)]    # free-dim chunks of 1250
RC = [(i * 128, min(128, RL - i * 128)) for i in range((RL + 127) // 128)]
NPAIR = (NT - 1) // 2       # 39 DoubleRow k-tile pairs; tile 78 done plain
GCP = 16                    # padded Gc pitch per k-tile (DoubleRow: step%16==0)
DMA_GROUP = 6               # AT k-tiles per batched DMA

USE_DR = True               # fp8 DoubleRow on the two A-passes
SHARE_LDW = os.environ.get("SHARELDW") == "1"


def _mm_group(nc, lhsT, mms, perf_mode=None):
    """Emit one explicit LDWEIGHTS + the group's matmuls flagged to reuse
    the loaded stationary (walrus skips their per-MM weight reload)."""
    if SHARE_LDW:
        nc.tensor.ldweights(lhsT, perf_mode=perf_mode)
    first = not SHARE_LDW
    for out, rhs, kw in mms:
        inst = nc.tensor.matmul(out, lhsT, rhs, perf_mode=perf_mode, **kw)
        if SHARE_LDW:
            try:
                inst.ldweights = False
            except AttributeError:
                inst.inst.ldweights = False
        first = False

_compiled = None


def _build():
    f32 = mybir.dt.float32
    bf16 = mybir.dt.bfloat16
    fp8 = mybir.dt.float8e4

    nc = bacc.Bacc("TRN2", target_bir_lowering=False, debug=False,
                   num_devices=NCORES)

    # ---- per-core inputs ----
    at_e = nc.dram_tensor("at", [N, RL], fp8, kind="ExternalInput").ap()
    xt_e = nc.dram_tensor("xt", [F_T, N], fp8, kind="ExternalInput").ap()
    xtl_e = nc.dram_tensor("xtl", [F_T, RL], bf16, kind="ExternalInput").ap()
    pt_e = nc.dram_tensor("pt", [F_P, RL], bf16, kind="ExternalInput").ap()
    w1x_e = nc.dram_tensor("w1x", [F_T, SEC + MC], fp8,
                           kind="ExternalInput").ap()
    ws2_e = nc.dram_tensor("ws2", [F_T, SEC], bf16, kind="ExternalInput").ap()
    bs_e = nc.dram_tensor("bs", [SEC, 1], f32, kind="ExternalInput").ap()
    wgs_e = nc.dram_tensor("wgs", [SEC, MC], bf16, kind="ExternalInput").ap()
    wgp_e = nc.dram_tensor("wgp", [F_P, MC], bf16, kind="ExternalInput").ap()
    was_e = nc.dram_tensor("was", [SEC, MC], bf16, kind="ExternalInput").ap()
    wax_e = nc.dram_tensor("wax", [F_T, MC], bf16, kind="ExternalInput").ap()
    wap_e = nc.dram_tensor("wap", [F_P, MC], bf16, kind="ExternalInput").ap()
    blb_e = nc.dram_tensor("blb", [MC, 512], f32, kind="ExternalInput").ap()
    out_e = nc.dram_tensor("out", [RL, MC], f32, kind="ExternalOutput").ap()

    with tile.TileContext(nc) as tc:
        with (
            tc.tile_pool(name="const", bufs=1) as cp,
            tc.tile_pool(name="big", bufs=1) as bigp,
            tc.tile_pool(name="work", bufs=1) as wp,
            tc.tile_pool(name="psum", bufs=1, space="PSUM") as pp,
            tc.tile_pool(name="dram", bufs=1, space="DRAM") as dp,
        ):
            # ---- constants / persistent tiles (small DMAs on gpsimd,
            # keeping the sync HWDGE queue free for the AT stream) ----
            w1x_s = cp.tile([F_T, SEC + MC], fp8)
            ws2_s = cp.tile([F_T, SEC], bf16)
            bs_s = cp.tile([SEC, 1], f32)
            wgs_s = cp.tile([SEC, MC], bf16)
            wgp_s = cp.tile([F_P, MC], bf16)
            was_s = cp.tile([SEC, MC], bf16)
            wax_s = cp.tile([F_T, MC], bf16)
            wap_s = cp.tile([F_P, MC], bf16)
            blb_s = cp.tile([MC, 512], f32)
            eye_s = cp.tile([MC, MC], f32)
            xtl_s = cp.tile([F_T, RL], bf16)
            pt_s = cp.tile([F_P, RL], bf16)
            xt_s = bigp.tile([F_T, N], fp8, name="xtfull")
            for dst, src in [(w1x_s, w1x_e), (ws2_s, ws2_e), (bs_s, bs_e),
                             (wgs_s, wgs_e), (wgp_s, wgp_e),
                             (was_s, was_e), (wax_s, wax_e), (wap_s, wap_e),
                             (blb_s, blb_e), (xtl_s, xtl_e), (pt_s, pt_e)]:
                nc.gpsimd.dma_start(dst[:], src[:])
            nc.sync.dma_start(xt_s[:], xt_e[:])
            masks.make_identity(nc, eye_s[:])

            g1_s = bigp.tile([128, NT * SEC], fp8, name="g1")
            g3x_s = bigp.tile([128, NT * GCP], fp8, name="g3x")
            sect = bigp.tile([SEC, RL], bf16, name="sect")
            gcf = bigp.tile([128, NT * GCP], fp8, name="gcf")
            combT = bigp.tile([MC, RL], f32, name="combT")
            at_s = bigp.tile([128, NT * RLP], fp8, name="atcache")
            at3 = at_s[:].rearrange("p (k i) -> p k i", i=RLP)
            g1v = g1_s[:].rearrange("p (k c) -> p k c", c=SEC)
            gcv = gcf[:].rearrange("p (k c) -> p k c", c=GCP)
            g3v = g3x_s[:].rearrange("p (k c) -> p k c", c=GCP)
            nc.gpsimd.memset(gcf[:], 0.0)
            nc.gpsimd.memset(g3x_s[:], 0.0)

            # ---- AT stream: batched DMAs on the sync HWDGE queue.
            # One junk matmul rides each group so the PE never idles a full
            # HAM MID window during the DMA phase (stays at 2.4 GHz). ----
            _dma_engs = [nc.sync, nc.scalar]
            for gi, g0 in enumerate(range(0, NT - 1, DMA_GROUP)):
                g1_ = min(g0 + DMA_GROUP, NT - 1)
                _dma_engs[gi % len(_dma_engs)].dma_start(
                    at3[:, g0:g1_, 0:RL],
                    at_e[g0 * 128:g1_ * 128, :].rearrange(
                        "(g p) i -> p g i", p=128))
                pw = pp.tile([128, 70], f32, tag="warm", bufs=1,
                             name=f"pw{gi}")
                nc.tensor.matmul(pw[:16, :16], at3[:, g0, 0:16],
                                 at3[:, g0, 0:16], start=True, stop=True)
            nc.sync.dma_start(at3[:KW[NT - 1], NT - 1, 0:RL],
                              at_e[(NT - 1) * 128:N, :])

            # ---- G1 = X Ws1 and G3x = X (0.5 v2 Wma_x) for all N rows ----
            for k in range(NT):
                kw = KW[k]
                pg = pp.tile([128, 70], f32, tag="small", bufs=3,
                             name=f"pg{k}")
                nc.tensor.matmul(pg[:kw, :SEC + MC],
                                 xt_s[:, k * 128:k * 128 + kw],
                                 w1x_s[:], start=True, stop=True)
                nc.vector.tensor_copy(g1_s[:kw, k * SEC:(k + 1) * SEC],
                                      pg[:kw, :SEC])
                nc.vector.tensor_copy(g3x_s[:kw, k * GCP:k * GCP + MC],
                                      pg[:kw, SEC:SEC + MC])

            # ---- pass 1: psum_s[ci] = X_loc Ws2 + (A @ G1)_loc^T ----
            ps_s = [pp.tile([SEC, 512], f32, tag="acc", bufs=3, name=f"ps{i}")
                    for i in range(3)]
            _mm_group(nc, ws2_s[:],
                      [(ps_s[ci][:, :w], xtl_s[:, off:off + w],
                        dict(start=True, stop=False))
                       for ci, (off, w) in enumerate(CHUNKS)])
            if USE_DR:
                for j in range(NPAIR):
                    _mm_group(nc, g1v[:, 2 * j:2 * j + 2, :],
                              [(ps_s[ci][:, :w],
                                at3[:, 2 * j:2 * j + 2, off:off + w],
                                dict(start=False, stop=False))
                               for ci, (off, w) in enumerate(CHUNKS)],
                              perf_mode=mybir.MatmulPerfMode.DoubleRow)
            else:
                for k in range(NT - 1):
                    for ci, (off, w) in enumerate(CHUNKS):
                        nc.tensor.matmul(ps_s[ci][:, :w],
                                         g1_s[:, k * SEC:(k + 1) * SEC],
                                         at3[:, k, off:off + w],
                                         start=False, stop=False)
            kl, kwl = NT - 1, KW[NT - 1]
            _mm_group(nc, g1_s[:kwl, kl * SEC:(kl + 1) * SEC],
                      [(ps_s[ci][:, :w], at3[:kwl, kl, off:off + w],
                        dict(start=False, stop=True))
                       for ci, (off, w) in enumerate(CHUNKS)])

            # ---- sec^T = relu(psum_s + bs) ----
            for ci, (off, w) in enumerate(CHUNKS):
                nc.scalar.activation(sect[:, off:off + w], ps_s[ci][:, :w],
                                     mybir.ActivationFunctionType.Relu,
                                     bias=bs_s[:, :])

            # ---- Gc_loc (natural [RL, 7], fp8) -> bounce -> AllGather ----
            gc_bounce = dp.tile([RL, MC], fp8, name="gc_bounce")
            gc_gather = dp.tile([N, MC], fp8, addr_space="Shared",
                                name="gc_gather")
            gcl = wp.tile([128, len(RC) * MC], fp8, name="gcl")
            gclv = gcl[:].rearrange("p (g c) -> p g c", c=MC)
            for ri, (o2, cw) in enumerate(RC):
                pgc = pp.tile([128, 70], f32, tag="small", bufs=3,
                              name=f"pgc{ri}")
                nc.tensor.matmul(pgc[:cw, :MC], sect[:, o2:o2 + cw],
                                 wgs_s[:], start=True, stop=False)
                nc.tensor.matmul(pgc[:cw, :MC], pt_s[:, o2:o2 + cw],
                                 wgp_s[:], start=False, stop=True)
                nc.vector.tensor_copy(gclv[:cw, ri, :], pgc[:cw, :MC])
            nc.sync.dma_start(
                gc_bounce[0:1152, :].rearrange("(g p) c -> p g c", p=128),
                gclv[:, 0:9, :])
            nc.sync.dma_start(gc_bounce[1152:RL, :], gclv[:98, 9, :])
            nc.gpsimd.collective_compute(
                "AllGather", mybir.AluOpType.bypass,
                ins=[gc_bounce[:].opt()], outs=[gc_gather[:].opt()],
                replica_groups=[list(range(NCORES))],
            )

            # ---- local additive terms into psum_main (reuses acc slots) ----
            ps_m = [pp.tile([SEC, 512], f32, tag="acc", bufs=3, name=f"pm{i}")
                    for i in range(3)]
            _mm_group(nc, was_s[:],
                      [(ps_m[ci][:MC, :w], sect[:, off:off + w],
                        dict(start=True, stop=False))
                       for ci, (off, w) in enumerate(CHUNKS)])
            _mm_group(nc, wax_s[:],
                      [(ps_m[ci][:MC, :w], xtl_s[:, off:off + w],
                        dict(start=False, stop=False))
                       for ci, (off, w) in enumerate(CHUNKS)])
            _mm_group(nc, wap_s[:],
                      [(ps_m[ci][:MC, :w], pt_s[:, off:off + w],
                        dict(start=False, stop=False))
                       for ci, (off, w) in enumerate(CHUNKS)])

            # ---- pass 2a: += (A @ G3x)^T — needs no gather, fills the
            # collective + skew window with PE work ----
            if USE_DR:
                for j in range(NPAIR):
                    _mm_group(nc, g3v[:, 2 * j:2 * j + 2, :],
                              [(ps_m[ci][:GCP, :w],
                                at3[:, 2 * j:2 * j + 2, off:off + w],
                                dict(start=False, stop=False))
                               for ci, (off, w) in enumerate(CHUNKS)],
                              perf_mode=mybir.MatmulPerfMode.DoubleRow)
            else:
                for k in range(NT - 1):
                    for ci, (off, w) in enumerate(CHUNKS):
                        nc.tensor.matmul(ps_m[ci][:MC, :w],
                                         g3x_s[:, k * GCP:k * GCP + MC],
                                         at3[:, k, off:off + w],
                                         start=False, stop=False)
            _mm_group(nc, g3x_s[:kwl, kl * GCP:kl * GCP + MC],
                      [(ps_m[ci][:MC, :w], at3[:kwl, kl, off:off + w],
                        dict(start=False, stop=False))
                       for ci, (off, w) in enumerate(CHUNKS)])

            # ---- load gathered Gc into SBUF k-tiles (chunked DMAs so
            # pass 2b can start as soon as the first k-tiles land) ----
            GCHUNK = 10
            for c0 in range(0, NT - 1, GCHUNK):
                c1 = min(c0 + GCHUNK, NT - 1)
                nc.sync.dma_start(
                    gcv[:, c0:c1, 0:MC],
                    gc_gather[c0 * 128:c1 * 128, :].rearrange(
                        "(k p) c -> p k c", p=128))
            nc.sync.dma_start(gcv[:KW[NT - 1], NT - 1, 0:MC],
                              gc_gather[(NT - 1) * 128:N, :])

            # ---- pass 2: += (A @ Gc)_loc^T, AT straight from SBUF ----
            if USE_DR:
                for j in range(NPAIR):
                    _mm_group(nc, gcv[:, 2 * j:2 * j + 2, :],
                              [(ps_m[ci][:GCP, :w],
                                at3[:, 2 * j:2 * j + 2, off:off + w],
                                dict(start=False, stop=False))
                               for ci, (off, w) in enumerate(CHUNKS)],
                              perf_mode=mybir.MatmulPerfMode.DoubleRow)
            else:
                for k in range(NT - 1):
                    for ci, (off, w) in enumerate(CHUNKS):
                        nc.tensor.matmul(ps_m[ci][:MC, :w],
                                         gcf[:, k * GCP:k * GCP + MC],
                                         at3[:, k, off:off + w],
                                         start=False, stop=False)
            _mm_group(nc, gcf[:kwl, kl * GCP:kl * GCP + MC],
                      [(ps_m[ci][:MC, :w], at3[:kwl, kl, off:off + w],
                        dict(start=False, stop=True))
                       for ci, (off, w) in enumerate(CHUNKS)])

            # ---- combT = psum_main + bl ----
            for ci, (off, w) in enumerate(CHUNKS):
                nc.vector.tensor_add(combT[:, off:off + w], ps_m[ci][:MC, :w],
                                     blb_s[:, :w])

            # ---- batched transpose + softmax (max|logit| ~ 32, no shift
            # needed: exp overflows only past ~88) ----
            ptl = pp.tile([128, 70], f32, tag="ptile", bufs=1, name="ptl")
            for ri, (o2, cw) in enumerate(RC):
                nc.tensor.transpose(ptl[:cw, ri * MC:(ri + 1) * MC],
                                    combT[:, o2:o2 + cw], eye_s[:])
            ex = wp.tile([128, 70], f32, name="ex")
            nrc = len(RC)
            nc.scalar.activation(ex[:, :], ptl[:, :],
                                 mybir.ActivationFunctionType.Exp)
            sm = wp.tile([128, nrc], f32, name="sm")
            nc.vector.tensor_reduce(
                sm[:, :], ex[:].rearrange("p (g c) -> p g c", c=MC),
                axis=mybir.AxisListType.X, op=mybir.AluOpType.add)
            rcp = wp.tile([128, nrc], f32, name="rcp")
            nc.vector.reciprocal(rcp[:, :], sm[:, :])
            ot = wp.tile([128, 70], f32, name="ot")
            nc.vector.tensor_mul(
                ot[:].rearrange("p (g c) -> p g c", c=MC),
                ex[:].rearrange("p (g c) -> p g c", c=MC),
                rcp[:].broadcast_to([128, nrc, MC]))
            nc.sync.dma_start(
                out_e[0:1152, :].rearrange("(g p) c -> p g c", p=128),
                ot[:].rearrange("p (g c) -> p g c", c=MC)[:, 0:9, :])
            nc.sync.dma_start(out_e[1152:RL, :], ot[:98, 63:70])

    nc.compile()
    return nc


def _get_compiled():
    global _compiled
    if _compiled is None:
        _compiled = _build()
    return _compiled


def kernel(temporal_features, A, path_features,
           Ws1, Ws2, bs, Wm2a, Wm2b, bm2, Wma, Wmb, bm, v1, v2,
           trace=False, tmpdir=None, trace_cores=None):
    nc = _get_compiled()

    X = np.asarray(temporal_features, np.float32)
    A = np.asarray(A, np.float32)
    P = np.asarray(path_features, np.float32)
    v1 = np.float32(v1)
    v2 = np.float32(v2)

    A8 = A.astype(FP8)
    xtf = np.ascontiguousarray(X.T)                        # [128, N] f32
    xt = xtf.astype(FP8)
    ptf = np.ascontiguousarray(P.T).astype(BF16)           # [64, N]

    ws1f = np.asarray(Ws1, np.float32)
    ws2 = np.asarray(Ws2, np.float32).astype(BF16)
    bs_in = np.asarray(bs, np.float32).reshape(SEC, 1)
    Wm2a = np.asarray(Wm2a, np.float32)
    Wm2b = np.asarray(Wm2b, np.float32)
    Wma = np.asarray(Wma, np.float32)
    Wmb = np.asarray(Wmb, np.float32)
    # pass-2 weights pre-scaled by 0.5*v (folds stack-mean + v-combine)
    w1x = np.concatenate([ws1f, 0.5 * v2 * Wma[:F_T]], axis=1).astype(FP8)
    wgs = (0.5 * v1 * Wm2a[:SEC]).astype(BF16)
    wgp = (0.5 * (v1 * Wm2a[SEC:] + v2 * Wma[F_T:])).astype(BF16)
    was = (0.5 * v1 * Wm2b[:SEC]).astype(BF16)
    wax = (0.5 * v2 * Wmb[:F_T]).astype(BF16)
    wap = (0.5 * (v1 * Wm2b[SEC:] + v2 * Wmb[F_T:])).astype(BF16)
    bl = 0.5 * (v2 * np.asarray(bm, np.float32) + v1 * np.asarray(bm2, np.float32))
    blb = np.tile(bl.reshape(MC, 1), (1, 512)).astype(np.float32)

    in_maps = []
    for c in range(NCORES):
        r0, r1 = c * RL, (c + 1) * RL
        in_maps.append({
            "at": np.ascontiguousarray(A8[r0:r1].T),
            "xt": xt,
            "xtl": np.ascontiguousarray(xtf[:, r0:r1]).astype(BF16),
            "pt": np.ascontiguousarray(ptf[:, r0:r1]),
            "w1x": w1x, "ws2": ws2, "bs": bs_in,
            "wgs": wgs, "wgp": wgp,
            "was": was, "wax": wax, "wap": wap,
            "blb": blb,
        })

    kwargs = {}
    if trace_cores is not None:
        kwargs["trace_cores"] = trace_cores
    res = run_bass_kernel_spmd(nc, in_maps, list(range(NCORES)),
                               trace=trace, tmpdir=tmpdir, **kwargs)
    out = np.concatenate([res.results[c]["out"] for c in range(NCORES)], axis=0)
    kernel.last_result = res
    return out


# revision 18
# speedup vs baseline: 1.4714x; 1.4714x over previous
"""Trainium2 Bass kernel for nn_AdaptativeGCN (gnn_message_passing).

Computation (reference):
    sec   = relu(A @ (X Ws1) + X Ws2 + bs)                 [N, 32]
    S     = [sec | P]                                      [N, 96]
    msec  = A @ (S Wm2a) + S Wm2b + bm2                    [N, 7]
    M     = [X | P]                                        [N, 192]
    main  = A @ (M Wma) + M Wmb + bm                       [N, 7]
    out   = softmax(0.5*(v2*main + v1*msec), axis=-1)      [N, 7]

Algebraic restructuring:
    0.5*(v2*main + v1*msec) = A @ Gc + L + bl  where
      Gc = 0.5*(v1*(S Wm2a) + v2*(M Wma))     [N, 7]   (one 7-col pass-2 product)
      L  = 0.5*(v1*(S Wm2b) + v2*(M Wmb))     [N, 7]   (local, no A)
      bl = 0.5*(v1*bm2 + v2*bm)               [7]

Distribution: row-shard A over 8 cores (1250 rows each). Host uploads
A[rows_c,:].T as fp8-e4m3 [10000, 1250] per core so the contraction dim lands
on SBUF partitions. The fp8 shard (11.9 MiB) is streamed from HBM once and
kept entirely resident in SBUF; pass 2 reuses it with zero HBM traffic. A@*
matmuls run fp8 DoubleRow (two k-planes per pass through the PE array). The
only cross-core dependency is Gc: one ~9 KB-per-core AllGather between the
passes. fp8 quantization error on the 10000-term contractions averages out:
host-simulated end-to-end rel err ~6e-4 vs the f32 reference.
"""

import sys
import types

import numpy as np


def _install_ntff_hook():
    """run_bass_kernel_spmd(trace=True) under axon needs antenv.axon_hooks,
    which the agent image lacks; register the ctypes-based hook ourselves."""
    try:
        from antenv.axon_hooks import get_axon_ntff_profile_hook  # noqa: F401
        return
    except ImportError:
        pass
    try:
        from trn_agent_boot.trn_boot import _ntff_profile_via_ctypes
        hook = _ntff_profile_via_ctypes('/opt/axon/libaxon_pjrt.so')
    except Exception:
        hook = None
    mod = types.ModuleType('antenv.axon_hooks')
    mod.get_axon_ntff_profile_hook = lambda: hook
    mod.set_axon_ntff_profile_hook = lambda h: None
    sys.modules['antenv.axon_hooks'] = mod


_install_ntff_hook()
if '/opt/trn_rl_repo' not in sys.path:
    sys.path.insert(0, '/opt/trn_rl_repo')

import ml_dtypes  # noqa: E402
import concourse.bacc as bacc  # noqa: E402
import concourse.mybir as mybir  # noqa: E402
from concourse import masks, tile  # noqa: E402
from concourse import bass_utils as _bu  # noqa: E402
from concourse.bass_utils import run_bass_kernel_spmd  # noqa: E402

import os  # noqa: E402

if os.environ.get("LDWOPT") == "1" and not getattr(_bu, "_ldw_patched", False):
    _orig_run_command = _bu.run_command

    def _patched_run_command(argv, **kwargs):
        argv = [a.replace("--enable-ldw-opt=false", "--enable-ldw-opt=true")
                if isinstance(a, str) else a for a in argv]
        return _orig_run_command(argv, **kwargs)

    _bu.run_command = _patched_run_command
    _bu._ldw_patched = True

BF16 = ml_dtypes.bfloat16
FP8 = ml_dtypes.float8_e4m3
NCORES = 8
N = 10000
F_T, F_P = 128, 64
SEC, MC = 32, 7
RL = N // NCORES            # local rows per core = 1250
RLP = 1280                  # padded SBUF pitch per AT k-tile (16B-aligned)
NT = (N + 127) // 128       # k-tiles over the contraction dim = 79
KW = [128] * (NT - 1) + [N - 128 * (NT - 1)]          # last = 16
CHUNKS = [(0, 512), (512, 512), (1024, RL - 1024)]    # free-dim chunks of 1250
RC = [(i * 128, min(128, RL - i * 128)) for i in range((RL + 127) // 128)]
NPAIR = (NT - 1) // 2       # 39 DoubleRow k-tile pairs; tile 78 done plain
GCP = 16                    # padded Gc pitch per k-tile (DoubleRow: step%16==0)
DMA_GROUP = 6               # AT k-tiles per batched DMA

USE_DR = True               # fp8 DoubleRow on the two A-passes
SHARE_LDW = os.environ.get("SHARELDW") == "1"


def _mm_group(nc, lhsT, mms, perf_mode=None):
    """Emit one explicit LDWEIGHTS + the group's matmuls flagged to reuse
    the loaded stationary (walrus skips their per-MM weight reload)."""
    if SHARE_LDW:
        nc.tensor.ldweights(lhsT, perf_mode=perf_mode)
    first = not SHARE_LDW
    for out, rhs, kw in mms:
        inst = nc.tensor.matmul(out, lhsT, rhs, perf_mode=perf_mode, **kw)
        if SHARE_LDW:
            try:
                inst.ldweights = False
            except AttributeError:
                inst.inst.ldweights = False
        first = False

_compiled = None


def _build():
    f32 = mybir.dt.float32
    bf16 = mybir.dt.bfloat16
    fp8 = mybir.dt.float8e4

    nc = bacc.Bacc("TRN2", target_bir_lowering=False, debug=False,
                   num_devices=NCORES)

    # ---- per-core inputs ----
    at_e = nc.dram_tensor("at", [N, RL], fp8, kind="ExternalInput").ap()
    xt_e = nc.dram_tensor("xt", [F_T, N], fp8, kind="ExternalInput").ap()
    xtl_e = nc.dram_tensor("xtl", [F_T, RL], bf16, kind="ExternalInput").ap()
    pt_e = nc.dram_tensor("pt", [F_P, RL], bf16, kind="ExternalInput").ap()
    w1x_e = nc.dram_tensor("w1x", [F_T, SEC + MC], fp8,
                           kind="ExternalInput").ap()
    ws2_e = nc.dram_tensor("ws2", [F_T, SEC], bf16, kind="ExternalInput").ap()
    bs_e = nc.dram_tensor("bs", [SEC, 1], f32, kind="ExternalInput").ap()
    wgs_e = nc.dram_tensor("wgs", [SEC, MC], bf16, kind="ExternalInput").ap()
    wgp_e = nc.dram_tensor("wgp", [F_P, MC], bf16, kind="ExternalInput").ap()
    was_e = nc.dram_tensor("was", [SEC, MC], bf16, kind="ExternalInput").ap()
    wax_e = nc.dram_tensor("wax", [F_T, MC], bf16, kind="ExternalInput").ap()
    wap_e = nc.dram_tensor("wap", [F_P, MC], bf16, kind="ExternalInput").ap()
    blb_e = nc.dram_tensor("blb", [MC, 512], f32, kind="ExternalInput").ap()
    out_e = nc.dram_tensor("out", [RL, MC], f32, kind="ExternalOutput").ap()

    with tile.TileContext(nc) as tc:
        with (
            tc.tile_pool(name="const", bufs=1) as cp,
            tc.tile_pool(name="big", bufs=1) as bigp,
            tc.tile_pool(name="work", bufs=1) as wp,
            tc.tile_pool(name="psum", bufs=1, space="PSUM") as pp,
            tc.tile_pool(name="dram", bufs=1, space="DRAM") as dp,
        ):
            # ---- constants / persistent tiles (small DMAs on gpsimd,
            # keeping the sync HWDGE queue free for the AT stream) ----
            w1x_s = cp.tile([F_T, SEC + MC], fp8)
            ws2_s = cp.tile([F_T, SEC], bf16)
            bs_s = cp.tile([SEC, 1], f32)
            wgs_s = cp.tile([SEC, MC], bf16)
            wgp_s = cp.tile([F_P, MC], bf16)
            was_s = cp.tile([SEC, MC], bf16)
            wax_s = cp.tile([F_T, MC], bf16)
            wap_s = cp.tile([F_P, MC], bf16)
            blb_s = cp.tile([MC, 512], f32)
            eye_s = cp.tile([MC, MC], f32)
            xtl_s = cp.tile([F_T, RL], bf16)
            pt_s = cp.tile([F_P, RL], bf16)
            xt_s = bigp.tile([F_T, N], fp8, name="xtfull")
            for dst, src in [(w1x_s, w1x_e), (ws2_s, ws2_e), (bs_s, bs_e),
                             (wgs_s, wgs_e), (wgp_s, wgp_e),
                             (was_s, was_e), (wax_s, wax_e), (wap_s, wap_e),
                             (blb_s, blb_e), (xtl_s, xtl_e), (pt_s, pt_e)]:
                nc.gpsimd.dma_start(dst[:], src[:])
            nc.sync.dma_start(xt_s[:], xt_e[:])
            masks.make_identity(nc, eye_s[:])

            g1_s = bigp.tile([128, NT * SEC], fp8, name="g1")
            g3x_s = bigp.tile([128, NT * GCP], fp8, name="g3x")
            sect = bigp.tile([SEC, RL], bf16, name="sect")
            gcf = bigp.tile([128, NT * GCP], fp8, name="gcf")
            combT = bigp.tile([MC, RL], f32, name="combT")
            at_s = bigp.tile([128, NT * RLP], fp8, name="atcache")
            at3 = at_s[:].rearrange("p (k i) -> p k i", i=RLP)
            g1v = g1_s[:].rearrange("p (k c) -> p k c", c=SEC)
            gcv = gcf[:].rearrange("p (k c) -> p k c", c=GCP)
            g3v = g3x_s[:].rearrange("p (k c) -> p k c", c=GCP)
            nc.gpsimd.memset(gcf[:], 0.0)
            nc.gpsimd.memset(g3x_s[:], 0.0)

            # ---- AT stream: batched DMAs on the sync HWDGE queue.
            # One junk matmul rides each group so the PE never idles a full
            # HAM MID window during the DMA phase (stays at 2.4 GHz). ----
            _dma_engs = [nc.sync]
            for gi, g0 in enumerate(range(0, NT - 1, DMA_GROUP)):
                g1_ = min(g0 + DMA_GROUP, NT - 1)
                _dma_engs[gi % len(_dma_engs)].dma_start(
                    at3[:, g0:g1_, 0:RL],
                    at_e[g0 * 128:g1_ * 128, :].rearrange(
                        "(g p) i -> p g i", p=128))
                pw = pp.tile([128, 70], f32, tag="warm", bufs=1,
                             name=f"pw{gi}")
                nc.tensor.matmul(pw[:16, :16], at3[:, g0, 0:16],
                                 at3[:, g0, 0:16], start=True, stop=True)
            nc.sync.dma_start(at3[:KW[NT - 1], NT - 1, 0:RL],
                              at_e[(NT - 1) * 128:N, :])

            # ---- G1 = X Ws1 and G3x = X (0.5 v2 Wma_x) for all N rows ----
            for k in range(NT):
                kw = KW[k]
                pg = pp.tile([128, 70], f32, tag="small", bufs=3,
                             name=f"pg{k}")
                nc.tensor.matmul(pg[:kw, :SEC + MC],
                                 xt_s[:, k * 128:k * 128 + kw],
                                 w1x_s[:], start=True, stop=True)
                nc.vector.tensor_copy(g1_s[:kw, k * SEC:(k + 1) * SEC],
                                      pg[:kw, :SEC])
                nc.vector.tensor_copy(g3x_s[:kw, k * GCP:k * GCP + MC],
                                      pg[:kw, SEC:SEC + MC])

            # ---- pass 1: psum_s[ci] = X_loc Ws2 + (A @ G1)_loc^T ----
            ps_s = [pp.tile([SEC, 512], f32, tag="acc", bufs=3, name=f"ps{i}")
                    for i in range(3)]
            _mm_group(nc, ws2_s[:],
                      [(ps_s[ci][:, :w], xtl_s[:, off:off + w],
                        dict(start=True, stop=False))
                       for ci, (off, w) in enumerate(CHUNKS)])
            if USE_DR:
                for j in range(NPAIR):
                    _mm_group(nc, g1v[:, 2 * j:2 * j + 2, :],
                              [(ps_s[ci][:, :w],
                                at3[:, 2 * j:2 * j + 2, off:off + w],
                                dict(start=False, stop=False))
                               for ci, (off, w) in enumerate(CHUNKS)],
                              perf_mode=mybir.MatmulPerfMode.DoubleRow)
            else:
                for k in range(NT - 1):
                    for ci, (off, w) in enumerate(CHUNKS):
                        nc.tensor.matmul(ps_s[ci][:, :w],
                                         g1_s[:, k * SEC:(k + 1) * SEC],
                                         at3[:, k, off:off + w],
                                         start=False, stop=False)
            kl, kwl = NT - 1, KW[NT - 1]
            _mm_group(nc, g1_s[:kwl, kl * SEC:(kl + 1) * SEC],
                      [(ps_s[ci][:, :w], at3[:kwl, kl, off:off + w],
                        dict(start=False, stop=True))
                       for ci, (off, w) in enumerate(CHUNKS)])

            # ---- sec^T = relu(psum_s + bs) ----
            for ci, (off, w) in enumerate(CHUNKS):
                nc.scalar.activation(sect[:, off:off + w], ps_s[ci][:, :w],
                                     mybir.ActivationFunctionType.Relu,
                                     bias=bs_s[:, :])

            # ---- Gc_loc (natural [RL, 7], fp8) -> bounce -> AllGather ----
            gc_bounce = dp.tile([RL, MC], fp8, name="gc_bounce")
            gc_gather = dp.tile([N, MC], fp8, addr_space="Shared",
                                name="gc_gather")
            gcl = wp.tile([128, len(RC) * MC], fp8, name="gcl")
            gclv = gcl[:].rearrange("p (g c) -> p g c", c=MC)
            for ri, (o2, cw) in enumerate(RC):
                pgc = pp.tile([128, 70], f32, tag="small", bufs=3,
                              name=f"pgc{ri}")
                nc.tensor.matmul(pgc[:cw, :MC], sect[:, o2:o2 + cw],
                                 wgs_s[:], start=True, stop=False)
                nc.tensor.matmul(pgc[:cw, :MC], pt_s[:, o2:o2 + cw],
                                 wgp_s[:], start=False, stop=True)
                nc.vector.tensor_copy(gclv[:cw, ri, :], pgc[:cw, :MC])
            nc.sync.dma_start(
                gc_bounce[0:1152, :].rearrange("(g p) c -> p g c", p=128),
                gclv[:, 0:9, :])
            nc.sync.dma_start(gc_bounce[1152:RL, :], gclv[:98, 9, :])
            nc.gpsimd.collective_compute(
                "AllGather", mybir.AluOpType.bypass,
                ins=[gc_bounce[:].opt()], outs=[gc_gather[:].opt()],
                replica_groups=[list(range(NCORES))],
            )

            # ---- local additive terms into psum_main (reuses acc slots) ----
            ps_m = [pp.tile([SEC, 512], f32, tag="acc", bufs=3, name=f"pm{i}")
                    for i in range(3)]
            _mm_group(nc, was_s[:],
                      [(ps_m[ci][:MC, :w], sect[:, off:off + w],
                        dict(start=True, stop=False))
                       for ci, (off, w) in enumerate(CHUNKS)])
            _mm_group(nc, wax_s[:],
                      [(ps_m[ci][:MC, :w], xtl_s[:, off:off + w],
                        dict(start=False, stop=False))
                       for ci, (off, w) in enumerate(CHUNKS)])
            _mm_group(nc, wap_s[:],
                      [(ps_m[ci][:MC, :w], pt_s[:, off:off + w],
                        dict(start=False, stop=False))
                       for ci, (off, w) in enumerate(CHUNKS)])

            # ---- pass 2a: += (A @ G3x)^T — needs no gather, fills the
            # collective + skew window with PE work ----
            if USE_DR:
                for j in range(NPAIR):
                    _mm_group(nc, g3v[:, 2 * j:2 * j + 2, :],
                              [(ps_m[ci][:GCP, :w],
                                at3[:, 2 * j:2 * j + 2, off:off + w],
                                dict(start=False, stop=False))
                               for ci, (off, w) in enumerate(CHUNKS)],
                              perf_mode=mybir.MatmulPerfMode.DoubleRow)
            else:
                for k in range(NT - 1):
                    for ci, (off, w) in enumerate(CHUNKS):
                        nc.tensor.matmul(ps_m[ci][:MC, :w],
                                         g3x_s[:, k * GCP:k * GCP + MC],
                                         at3[:, k, off:off + w],
                                         start=False, stop=False)
            _mm_group(nc, g3x_s[:kwl, kl * GCP:kl * GCP + MC],
                      [(ps_m[ci][:MC, :w], at3[:kwl, kl, off:off + w],
                        dict(start=False, stop=False))
                       for ci, (off, w) in enumerate(CHUNKS)])

            # ---- load gathered Gc into SBUF k-tiles (chunked DMAs so
            # pass 2b can start as soon as the first k-tiles land) ----
            GCHUNK = 10
            for c0 in range(0, NT - 1, GCHUNK):
                c1 = min(c0 + GCHUNK, NT - 1)
                nc.sync.dma_start(
                    gcv[:, c0:c1, 0:MC],
                    gc_gather[c0 * 128:c1 * 128, :].rearrange(
                        "(k p) c -> p k c", p=128))
            nc.sync.dma_start(gcv[:KW[NT - 1], NT - 1, 0:MC],
                              gc_gather[(NT - 1) * 128:N, :])

            # ---- pass 2: += (A @ Gc)_loc^T, AT straight from SBUF ----
            if USE_DR:
                for j in range(NPAIR):
                    _mm_group(nc, gcv[:, 2 * j:2 * j + 2, :],
                              [(ps_m[ci][:GCP, :w],
                                at3[:, 2 * j:2 * j + 2, off:off + w],
                                dict(start=False, stop=False))
                               for ci, (off, w) in enumerate(CHUNKS)],
                              perf_mode=mybir.MatmulPerfMode.DoubleRow)
            else:
                for k in range(NT - 1):
                    for ci, (off, w) in enumerate(CHUNKS):
                        nc.tensor.matmul(ps_m[ci][:MC, :w],
                                         gcf[:, k * GCP:k * GCP + MC],
                                         at3[:, k, off:off + w],
                                         start=False, stop=False)
            _mm_group(nc, gcf[:kwl, kl * GCP:kl * GCP + MC],
                      [(ps_m[ci][:MC, :w], at3[:kwl, kl, off:off + w],
                        dict(start=False, stop=True))
                       for ci, (off, w) in enumerate(CHUNKS)])

            # ---- combT = psum_main + bl ----
            for ci, (off, w) in enumerate(CHUNKS):
                nc.vector.tensor_add(combT[:, off:off + w], ps_m[ci][:MC, :w],
                                     blb_s[:, :w])

            # ---- batched transpose + softmax (max|logit| ~ 32, no shift
            # needed: exp overflows only past ~88) ----
            ptl = pp.tile([128, 70], f32, tag="ptile", bufs=1, name="ptl")
            for ri, (o2, cw) in enumerate(RC):
                nc.tensor.transpose(ptl[:cw, ri * MC:(ri + 1) * MC],
                                    combT[:, o2:o2 + cw], eye_s[:])
            ex = wp.tile([128, 70], f32, name="ex")
            nrc = len(RC)
            nc.scalar.activation(ex[:, :], ptl[:, :],
                                 mybir.ActivationFunctionType.Exp)
            sm = wp.tile([128, nrc], f32, name="sm")
            nc.vector.tensor_reduce(
                sm[:, :], ex[:].rearrange("p (g c) -> p g c", c=MC),
                axis=mybir.AxisListType.X, op=mybir.AluOpType.add)
            rcp = wp.tile([128, nrc], f32, name="rcp")
            nc.vector.reciprocal(rcp[:, :], sm[:, :])
            ot = wp.tile([128, 70], f32, name="ot")
            nc.vector.tensor_mul(
                ot[:].rearrange("p (g c) -> p g c", c=MC),
                ex[:].rearrange("p (g c) -> p g c", c=MC),
                rcp[:].broadcast_to([128, nrc, MC]))
            nc.sync.dma_start(
                out_e[0:1152, :].rearrange("(g p) c -> p g c", p=128),
                ot[:].rearrange("p (g c) -> p g c", c=MC)[:, 0:9, :])
            nc.sync.dma_start(out_e[1152:RL, :], ot[:98, 63:70])

    nc.compile()
    return nc


def _get_compiled():
    global _compiled
    if _compiled is None:
        _compiled = _build()
    return _compiled


def kernel(temporal_features, A, path_features,
           Ws1, Ws2, bs, Wm2a, Wm2b, bm2, Wma, Wmb, bm, v1, v2,
           trace=False, tmpdir=None, trace_cores=None):
    nc = _get_compiled()

    X = np.asarray(temporal_features, np.float32)
    A = np.asarray(A, np.float32)
    P = np.asarray(path_features, np.float32)
    v1 = np.float32(v1)
    v2 = np.float32(v2)

    A8 = A.astype(FP8)
    xtf = np.ascontiguousarray(X.T)                        # [128, N] f32
    xt = xtf.astype(FP8)
    ptf = np.ascontiguousarray(P.T).astype(BF16)           # [64, N]

    ws1f = np.asarray(Ws1, np.float32)
    ws2 = np.asarray(Ws2, np.float32).astype(BF16)
    bs_in = np.asarray(bs, np.float32).reshape(SEC, 1)
    Wm2a = np.asarray(Wm2a, np.float32)
    Wm2b = np.asarray(Wm2b, np.float32)
    Wma = np.asarray(Wma, np.float32)
    Wmb = np.asarray(Wmb, np.float32)
    # pass-2 weights pre-scaled by 0.5*v (folds stack-mean + v-combine)
    w1x = np.concatenate([ws1f, 0.5 * v2 * Wma[:F_T]], axis=1).astype(FP8)
    wgs = (0.5 * v1 * Wm2a[:SEC]).astype(BF16)
    wgp = (0.5 * (v1 * Wm2a[SEC:] + v2 * Wma[F_T:])).astype(BF16)
    was = (0.5 * v1 * Wm2b[:SEC]).astype(BF16)
    wax = (0.5 * v2 * Wmb[:F_T]).astype(BF16)
    wap = (0.5 * (v1 * Wm2b[SEC:] + v2 * Wmb[F_T:])).astype(BF16)
    bl = 0.5 * (v2 * np.asarray(bm, np.float32) + v1 * np.asarray(bm2, np.float32))
    blb = np.tile(bl.reshape(MC, 1), (1, 512)).astype(np.float32)

    in_maps = []
    for c in range(NCORES):
        r0, r1 = c * RL, (c + 1) * RL
        in_maps.append({
            "at": np.ascontiguousarray(A8[r0:r1].T),
            "xt": xt,
            "xtl": np.ascontiguousarray(xtf[:, r0:r1]).astype(BF16),
            "pt": np.ascontiguousarray(ptf[:, r0:r1]),
            "w1x": w1x, "ws2": ws2, "bs": bs_in,
            "wgs": wgs, "wgp": wgp,
            "was": was, "wax": wax, "wap": wap,
            "blb": blb,
        })

    kwargs = {}
    if trace_cores is not None:
        kwargs["trace_cores"] = trace_cores
    res = run_bass_kernel_spmd(nc, in_maps, list(range(NCORES)),
                               trace=trace, tmpdir=tmpdir, **kwargs)
    out = np.concatenate([res.results[c]["out"] for c in range(NCORES)], axis=0)
    kernel.last_result = res
    return out
